# revision 32
# baseline (speedup 1.0000x reference)
"""Trainium2 Bass kernel for CorpusSupportSets RBF tangent-field.

Math per sample row i (dim 768), one-hot mask selects dipole k:
    t_j  = z . s_j                      (unit z, unit s_j)
    m_j  = a_j g_j e^{-g_j(2-2t_j)} = C_j exp(2 g_j t_j),  C_j = a_j g_j e^{-2 g_j}
    beta = -(m0 t0 + m1 t1)
    p    = beta z + m0 s0 + m1 s1
    |p|^2 = m0^2 + m1^2 - beta^2 + 2 m0 m1 (s0.s1)
    out  = p / |p|

Sharding: data-parallel over batch across 8 cores (2048 rows each).
Host prep (dtype/layout only + per-table-row constants): z -> bf16;
mask -> f16 scaled by column index (one-hot * k, exact in f16); table
rows [s0|s1|C0|C1|2g0|2g1|c01|pad] in bf16 (1664 cols = 3328B, 256B
multiple for dma_gather); output computed in bf16, upcast on host.

Per-sample row selection uses indirect DMA row gathers with u32
per-partition offsets computed from the scaled-mask reduction.
"""
import sys

for _p in ("/opt/trn_rl_repo",):
    if _p not in sys.path:
        sys.path.insert(0, _p)

import numpy as np

import concourse.bass as bass
import concourse.tile as tile
from concourse import mybir
from concourse.bass_utils import run_bass_kernel_spmd
from concourse.vector_clock import ScopedClock

# ---------------------------------------------------------------------------
# Workaround: this walrus build only accepts ONE semaphore wait per
# instruction; the TileContext exit drain accumulates one wait per live
# semaphore lane.  Split overflow waits onto trailing sync-engine NOPs.
_MAX_WAITS = 1


def _split_waits(nc, inst):
    si = inst.sync_info
    if si is None:
        return
    waits = list(si.on_wait)
    if len(waits) <= _MAX_WAITS:
        return
    inst.sync_info = mybir.SyncInfo(
        on_wait=waits[:_MAX_WAITS], on_update=list(si.on_update)
    )
    for i in range(_MAX_WAITS, len(waits), _MAX_WAITS):
        nop = nc.sync.nop(nofuse=True, hint="drain_wait_overflow")
        nop.ins.sync_info = mybir.SyncInfo(
            on_wait=waits[i : i + _MAX_WAITS], on_update=[]
        )


def _patched_drain_and_barrier(self, tick_clock, wait_clock):
    drain_inst = self.nc.sync.drain()
    wait_clock.add_sem_waits(
        drain_inst.ins, ScopedClock({None: tick_clock.global_clock})
    )
    _split_waits(self.nc, drain_inst.ins)
    self.nc.all_engine_barrier()
    assert self.sems is not None
    popped = self.nc._tile_sem_poison_stack.pop()
    assert popped is self._sem_poison
    self.nc.clear_and_free_semaphores(list(self.sems.allocated().values()))
    self.nc.all_engine_barrier()


_orig_commit = tile.TileContext._commit_instruction


def _patched_commit(self, inst, lazy_reg_writes=True):
    si = getattr(inst, "sync_info", None)
    if (
        si is not None
        and si.on_wait
        and len(si.on_wait) > _MAX_WAITS
        and inst.engine != mybir.EngineType.Unassigned
    ):
        waits = list(si.on_wait)
        inst.sync_info = mybir.SyncInfo(
            on_wait=waits[:_MAX_WAITS], on_update=list(si.on_update)
        )
        for _i, _w in enumerate(waits[_MAX_WAITS:]):
            nop = mybir.InstNoOp(
                name=f"{inst.name}_w{_i}",
                engine=inst.engine,
                sync_info=mybir.SyncInfo(on_wait=[_w], on_update=[]),
                bass_nofuse=True,
            )
            self._add_instruction(nop)
    return _orig_commit(self, inst, lazy_reg_writes)


tile.TileContext._drain_and_barrier = _patched_drain_and_barrier
tile.TileContext._commit_instruction = _patched_commit

# ---------------------------------------------------------------------------
BS, K, DIM = 16384, 1000, 768
NCORES = 8
ROWS = BS // NCORES  # 2048 rows per core
P = 128
NT = ROWS // P  # 16 tiles of 128 rows
GRP = 4  # tiles per group
NG = NT // GRP  # 4 groups
KP = 125  # packed (block-summed) scaled-mask width: 1000 = 8 * 125
TBL_W = 2 * DIM + 8  # 1544 bf16 cols = 3088B per row
# param columns inside a table row
PC = 2 * DIM  # C0, C1, 2g0, 2g1, c01 start here
F32 = mybir.dt.float32
BF16 = mybir.dt.bfloat16
F16 = mybir.dt.float16
U32 = mybir.dt.uint32


def build_nc(rows=ROWS):
    NT = rows // P
    NG = NT // GRP
    OP = mybir.AluOpType
    AT = mybir.ActivationFunctionType
    nc = bass.Bass()
    zin = nc.dram_tensor("zin", [rows, DIM], BF16, kind="ExternalInput")
    mkv = nc.dram_tensor("mkv", [rows, KP], F16, kind="ExternalInput")
    tbl = nc.dram_tensor("tbl", [K, TBL_W], BF16, kind="ExternalInput")
    eye = nc.dram_tensor("eye", [P, P], BF16, kind="ExternalInput")
    out = nc.dram_tensor("out", [rows, DIM], BF16, kind="ExternalOutput")

    with tile.TileContext(nc) as tc:
        with (
            tc.tile_pool(name="zp", bufs=3) as zp,
            tc.tile_pool(name="mkp", bufs=2) as mkp,
            tc.tile_pool(name="selp", bufs=3) as selp,
            tc.tile_pool(name="outp", bufs=2) as outp,
            tc.tile_pool(name="outp", bufs=2) as outp,
            tc.tile_pool(name="scrap", bufs=4) as scrp,
            tc.tile_pool(name="ascrap", bufs=2) as ascrp,
            tc.tile_pool(name="tiny", bufs=80) as tinyp,
            tc.tile_pool(name="gacc", bufs=12) as gaccp,
        ):


            def phase1(g):
                r0, r1 = g * GRP * P, (g + 1) * GRP * P
                c0 = g * GRP
                mk_g = mkp.tile([P, GRP, KP], F16, name="mk_g", tag="mk")
                nc.sync.dma_start(
                    out=mk_g[:], in_=mkv[r0:r1].rearrange("(n p) c -> p n c", p=P)
                )
                z_g = zp.tile([P, GRP, DIM], BF16, name="z_g", tag="z")
                for n in range(GRP):
                    zr = r0 + n * P
                    nc.gpsimd.dma_start(out=z_g[:, n, :], in_=zin[zr : zr + P])
                # idx for the whole group in one cheap reduce (exact)
                nc.vector.tensor_reduce(
                    out=idxf[:, c0 : c0 + GRP],
                    in_=mk_g[:],
                    axis=mybir.AxisListType.X,
                    op=OP.add,
                )
                # convert group idx to u32 gather offsets
                nc.vector.tensor_copy(
                    out=idxu[:, c0 : c0 + GRP], in_=idxf[:, c0 : c0 + GRP]
                )
                # gather table rows (one indirect DMA per tile)
                sel = selp.tile([P, GRP, TBL_W], BF16, name="sel", tag="sel")
                for n in range(GRP):
                    nc.gpsimd.indirect_dma_start(
                        out=sel[:, n, :],
                        out_offset=None,
                        in_=tbl[:],
                        in_offset=bass.IndirectOffsetOnAxis(
                            ap=idxu[:, c0 + n : c0 + n + 1], axis=0
                        ),
                    )
                return dict(g=g, z_g=z_g, sel=sel)

            def phaseB(st):
                # t_j = z . s_j. Most via one DVE op (mult + free-axis accum);
                # some pole-1 dots via GpSimd add + ACT square-accum
                # (q = |z+s|^2 = 2+2t), fixed up to t afterwards on V.
                g, z_g, sel = st["g"], st["z_g"], st["sel"]
                c0 = g * GRP
                noff = 2 if g < 2 else 1
                for n in range(GRP):
                    j = c0 + n
                    for pole in range(2):
                        if pole == 1 and n < noff:
                            wscr = scrp.tile([P, DIM], BF16, name="wscr", tag="scr")
                            nc.gpsimd.tensor_tensor(
                                out=wscr[:],
                                in0=z_g[:, n, :],
                                in1=sel[:, n, DIM : 2 * DIM],
                                op=OP.add,
                            )
                            qscr = scrp.tile([P, DIM], BF16, name="qscr", tag="scr")
                            nc.scalar.activation(
                                out=qscr[:],
                                in_=wscr[:],
                                func=AT.Square,
                                accum_out=t_a[:, 2 * j + 1 : 2 * j + 2],
                            )
                            nc.vector.tensor_scalar(
                                out=t_a[:, 2 * j + 1 : 2 * j + 2],
                                in0=t_a[:, 2 * j + 1 : 2 * j + 2],
                                scalar1=0.5, scalar2=-1.0, op0=OP.mult, op1=OP.add,
                            )
                        else:
                            pscr = scrp.tile([P, DIM], BF16, name="pscr", tag="scr")
                            nc.vector.scalar_tensor_tensor(
                                out=pscr[:],
                                in0=z_g[:, n, :],
                                scalar=1.0,
                                in1=sel[:, n, pole * DIM : (pole + 1) * DIM],
                                op0=OP.mult,
                                op1=OP.mult,
                                accum_out=t_a[:, 2 * j + pole : 2 * j + pole + 1],
                            )

            def phase2(st, last=False):
                g = st["g"]
                r0, r1 = g * GRP * P, (g + 1) * GRP * P
                c0 = g * GRP
                z_g, sel = st["z_g"], st["sel"]
                tg = t_a[:, 8 * g : 8 * (g + 1)].rearrange(
                    "p (c t) -> p c t", t=2
                )  # [P, GRP, 2] f32
                # small per-sample math on [P, GRP(,2)] tiles
                selC = sel[:, :, PC : PC + 2]  # bf16 [P,GRP,2]
                selG2 = sel[:, :, PC + 2 : PC + 4]
                selc01 = sel[:, :, PC + 4 : PC + 5].rearrange("p c o -> p (c o)")
                u = tinyp.tile([P, GRP, 2], F32, name="u", tag="tiny")
                nc.vector.tensor_tensor(out=u[:], in0=selG2, in1=tg, op=OP.mult)
                e = tinyp.tile([P, GRP, 2], F32, name="e", tag="tiny")
                nc.scalar.activation(out=e[:], in_=u[:], func=AT.Exp)
                m = tinyp.tile([P, GRP, 2], F32, name="m", tag="tiny")
                nc.vector.tensor_tensor(out=m[:], in0=selC, in1=e[:], op=OP.mult)
                h = tinyp.tile([P, GRP, 2], F32, name="h", tag="tiny")
                nc.vector.tensor_tensor(out=h[:], in0=m[:], in1=tg, op=OP.mult)
                beta = tinyp.tile([P, GRP], F32, name="beta", tag="tiny")
                nc.vector.scalar_tensor_tensor(
                    out=beta[:], in0=h[:, :, 0], scalar=-1.0, in1=h[:, :, 1],
                    op0=OP.mult, op1=OP.subtract,
                )
                m2 = tinyp.tile([P, GRP, 2], F32, name="m2", tag="tiny")
                nc.vector.tensor_tensor(out=m2[:], in0=m[:], in1=m[:], op=OP.mult)
                s2 = tinyp.tile([P, GRP], F32, name="s2", tag="tiny")
                nc.vector.tensor_tensor(
                    out=s2[:], in0=m2[:, :, 0], in1=m2[:, :, 1], op=OP.add
                )
                mm = tinyp.tile([P, GRP], F32, name="mm", tag="tiny")
                nc.vector.tensor_tensor(
                    out=mm[:], in0=m[:, :, 0], in1=m[:, :, 1], op=OP.mult
                )
                v = tinyp.tile([P, GRP], F32, name="v", tag="tiny")
                nc.vector.tensor_tensor(out=v[:], in0=mm[:], in1=selc01, op=OP.mult)
                b2 = tinyp.tile([P, GRP], F32, name="b2", tag="tiny")
                nc.vector.tensor_tensor(out=b2[:], in0=beta[:], in1=beta[:], op=OP.mult)
                w = tinyp.tile([P, GRP], F32, name="w", tag="tiny")
                nc.vector.scalar_tensor_tensor(
                    out=w[:], in0=v[:], scalar=2.0, in1=b2[:],
                    op0=OP.mult, op1=OP.subtract,
                )
                pn = tinyp.tile([P, GRP], F32, name="pn", tag="tiny")
                nc.vector.tensor_tensor(out=pn[:], in0=s2[:], in1=w[:], op=OP.add)
                # r = 1/|p| = exp(-0.5 ln pn); ln+exp share one ACT table
                ll = tinyp.tile([P, GRP], F32, name="ll", tag="tiny")
                nc.scalar.activation(out=ll[:], in_=pn[:], func=AT.Ln)
                rr = tinyp.tile([P, GRP], F32, name="rr", tag="tiny")
                nc.scalar.activation(out=rr[:], in_=ll[:], func=AT.Exp, scale=-0.5)
                bp = tinyp.tile([P, GRP], F32, name="bp", tag="tiny")
                nc.vector.tensor_tensor(out=bp[:], in0=beta[:], in1=rr[:], op=OP.mult)
                m0p = tinyp.tile([P, GRP], F32, name="m0p", tag="tiny")
                nc.vector.tensor_tensor(
                    out=m0p[:], in0=m[:, :, 0], in1=rr[:], op=OP.mult
                )
                m1p = tinyp.tile([P, GRP], F32, name="m1p", tag="tiny")
                nc.vector.tensor_tensor(
                    out=m1p[:], in0=m[:, :, 1], in1=rr[:], op=OP.mult
                )
                # p = diag(bp) z + diag(m0p) s0 + diag(m1p) s1 on the PE,
                # accumulated in PSUM f32, copied to SBUF bf16, DMA'd out
                og = outp.tile([P, GRP, DIM], BF16, name="og", tag="og")
                for n in range(GRP):
                    dB = diagp.tile([P, P], BF16, name="dB", tag="diag")
                    nc.vector.tensor_scalar(
                        out=dB[:], in0=eyeb[:], scalar1=bp[:, n : n + 1],
                        scalar2=None, op0=OP.mult,
                    )
                    d0 = diagp.tile([P, P], BF16, name="d0", tag="diag")
                    nc.scalar.activation(
                        out=d0[:], in_=eyeb[:], func=AT.Copy,
                        scale=m0p[:, n : n + 1],
                    )
                    d1 = diagp.tile([P, P], BF16, name="d1", tag="diag")
                    nc.scalar.activation(
                        out=d1[:], in_=eyeb[:], func=AT.Copy,
                        scale=m1p[:, n : n + 1],
                    )
                    pp = psump.tile([P, 2, 512], F32, name="pp", tag="psum")
                    for h2 in range(2):
                        lo = h2 * 384
                        nc.tensor.matmul(
                            pp[:, h2, 0:384], dB[:], z_g[:, n, lo : lo + 384],
                            start=True, stop=False,
                        )
                        nc.tensor.matmul(
                            pp[:, h2, 0:384], d0[:], sel[:, n, lo : lo + 384],
                            start=False, stop=False,
                        )
                        nc.tensor.matmul(
                            pp[:, h2, 0:384], d1[:],
                            sel[:, n, DIM + lo : DIM + lo + 384],
                            start=False, stop=True,
                        )
                    nc.scalar.copy(out=og[:, n, :], in_=pp[:, :, 0:384])
                nc.sync.dma_start(
                    out=out[r0 : r0 + GRP * P].rearrange("(n p) c -> p n c", p=P),
                    in_=og[:],
                )

            sts = {}
            sts[0] = phase1(0)
            sts[1] = phase1(1)
            nc.sync.dma_start(out=eyeb[:], in_=eye[:])
            sts[2] = phase1(2)
            phaseB(sts[0])
            phase2(sts[0])
            sts[3] = phase1(3)
            phaseB(sts[1])
            phase2(sts[1])
            phaseB(sts[2])
            phase2(sts[2])
            phaseB(sts[3])
            phase2(sts[3], last=True)
    return nc


_NC_CACHE = None


def _get_nc():
    global _NC_CACHE
    if _NC_CACHE is None:
        _NC_CACHE = build_nc()
    return _NC_CACHE


def build_in_maps(inputs):
    import ml_dtypes

    z = np.asarray(inputs["z"], dtype=np.float32)
    mask = np.asarray(inputs["support_sets_mask"], dtype=np.float32)
    S = np.asarray(inputs["SUPPORT_SETS"], dtype=np.float32)
    A = np.asarray(inputs["ALPHAS"], dtype=np.float32)
    LG = np.asarray(inputs["LOGGAMMA"], dtype=np.float32)

    zb = z.astype(ml_dtypes.bfloat16)
    # scaled one-hot (value = column index), block-summed to 125 columns;
    # exact because each row has a single nonzero
    mkv = (
        (mask * np.arange(K, dtype=np.float32)[None, :])
        .reshape(BS, 8, KP)
        .sum(axis=1)
        .astype(np.float16)
    )
    # table rows: [s0 | s1 | C0 C1 2g0 2g1 c01 | pad]
    g = np.exp(LG)  # [K,2]
    C = A * g * np.exp(-2.0 * g)
    c01 = np.sum(S[:, :DIM] * S[:, DIM:], axis=1, keepdims=True)
    tblf = np.zeros((K, TBL_W), dtype=np.float32)
    tblf[:, : 2 * DIM] = S
    tblf[:, PC : PC + 2] = C
    tblf[:, PC + 2 : PC + 4] = 2.0 * g
    tblf[:, PC + 4 : PC + 5] = c01
    tbl = tblf.astype(ml_dtypes.bfloat16)
    eye = np.eye(P, dtype=np.float32).astype(ml_dtypes.bfloat16)

    return [
        {
            "zin": np.ascontiguousarray(zb[c * ROWS : (c + 1) * ROWS]),
            "mkv": np.ascontiguousarray(mkv[c * ROWS : (c + 1) * ROWS]),
            "tbl": tbl,
            "eye": eye,
        }
        for c in range(NCORES)
    ]


def kernel(support_sets_mask, z, SUPPORT_SETS, ALPHAS, LOGGAMMA):
    in_maps = build_in_maps(
        dict(
            support_sets_mask=support_sets_mask, z=z,
            SUPPORT_SETS=SUPPORT_SETS, ALPHAS=ALPHAS, LOGGAMMA=LOGGAMMA,
        )
    )
    nc = _get_nc()
    res = run_bass_kernel_spmd(nc, in_maps, list(range(NCORES)))
    return np.concatenate(
        [res.results[c]["out"] for c in range(NCORES)], axis=0
    ).astype(np.float32)


# revision 33
# speedup vs baseline: 1.1855x; 1.1855x over previous
"""Trainium2 Bass kernel for CorpusSupportSets RBF tangent-field.

Math per sample row i (dim 768), one-hot mask selects dipole k:
    t_j  = z . s_j                      (unit z, unit s_j)
    m_j  = a_j g_j e^{-g_j(2-2t_j)} = C_j exp(2 g_j t_j),  C_j = a_j g_j e^{-2 g_j}
    beta = -(m0 t0 + m1 t1)
    p    = beta z + m0 s0 + m1 s1
    |p|^2 = m0^2 + m1^2 - beta^2 + 2 m0 m1 (s0.s1)
    out  = p / |p|

Sharding: data-parallel over batch across 8 cores (2048 rows each).
Host prep (dtype/layout only + per-table-row constants): z -> bf16;
mask -> f16 scaled by column index (one-hot * k, exact in f16); table
rows [s0|s1|C0|C1|2g0|2g1|c01|pad] in bf16 (1664 cols = 3328B, 256B
multiple for dma_gather); output computed in bf16, upcast on host.

Per-sample row selection uses indirect DMA row gathers with u32
per-partition offsets computed from the scaled-mask reduction.
"""
import sys

for _p in ("/opt/trn_rl_repo",):
    if _p not in sys.path:
        sys.path.insert(0, _p)

import numpy as np

import concourse.bass as bass
import concourse.tile as tile
from concourse import mybir
from concourse.bass_utils import run_bass_kernel_spmd
from concourse.vector_clock import ScopedClock

# ---------------------------------------------------------------------------
# Workaround: this walrus build only accepts ONE semaphore wait per
# instruction; the TileContext exit drain accumulates one wait per live
# semaphore lane.  Split overflow waits onto trailing sync-engine NOPs.
_MAX_WAITS = 1


def _split_waits(nc, inst):
    si = inst.sync_info
    if si is None:
        return
    waits = list(si.on_wait)
    if len(waits) <= _MAX_WAITS:
        return
    inst.sync_info = mybir.SyncInfo(
        on_wait=waits[:_MAX_WAITS], on_update=list(si.on_update)
    )
    for i in range(_MAX_WAITS, len(waits), _MAX_WAITS):
        nop = nc.sync.nop(nofuse=True, hint="drain_wait_overflow")
        nop.ins.sync_info = mybir.SyncInfo(
            on_wait=waits[i : i + _MAX_WAITS], on_update=[]
        )


def _patched_drain_and_barrier(self, tick_clock, wait_clock):
    drain_inst = self.nc.sync.drain()
    wait_clock.add_sem_waits(
        drain_inst.ins, ScopedClock({None: tick_clock.global_clock})
    )
    _split_waits(self.nc, drain_inst.ins)
    self.nc.all_engine_barrier()
    assert self.sems is not None
    popped = self.nc._tile_sem_poison_stack.pop()
    assert popped is self._sem_poison
    self.nc.clear_and_free_semaphores(list(self.sems.allocated().values()))
    self.nc.all_engine_barrier()


_orig_commit = tile.TileContext._commit_instruction


def _patched_commit(self, inst, lazy_reg_writes=True):
    si = getattr(inst, "sync_info", None)
    if (
        si is not None
        and si.on_wait
        and len(si.on_wait) > _MAX_WAITS
        and inst.engine != mybir.EngineType.Unassigned
    ):
        waits = list(si.on_wait)
        inst.sync_info = mybir.SyncInfo(
            on_wait=waits[:_MAX_WAITS], on_update=list(si.on_update)
        )
        for _i, _w in enumerate(waits[_MAX_WAITS:]):
            nop = mybir.InstNoOp(
                name=f"{inst.name}_w{_i}",
                engine=inst.engine,
                sync_info=mybir.SyncInfo(on_wait=[_w], on_update=[]),
                bass_nofuse=True,
            )
            self._add_instruction(nop)
    return _orig_commit(self, inst, lazy_reg_writes)


tile.TileContext._drain_and_barrier = _patched_drain_and_barrier
tile.TileContext._commit_instruction = _patched_commit

# ---------------------------------------------------------------------------
BS, K, DIM = 16384, 1000, 768
NCORES = 8
ROWS = BS // NCORES  # 2048 rows per core
P = 128
NT = ROWS // P  # 16 tiles of 128 rows
GRP = 4  # tiles per group
NG = NT // GRP  # 4 groups
KP = 125  # packed (block-summed) scaled-mask width: 1000 = 8 * 125
TBL_W = 2 * DIM + 8  # 1544 bf16 cols = 3088B per row
# param columns inside a table row
PC = 2 * DIM  # C0, C1, 2g0, 2g1, c01 start here
F32 = mybir.dt.float32
BF16 = mybir.dt.bfloat16
F16 = mybir.dt.float16
U32 = mybir.dt.uint32


def build_nc(rows=ROWS):
    NT = rows // P
    NG = NT // GRP
    OP = mybir.AluOpType
    AT = mybir.ActivationFunctionType
    nc = bass.Bass()
    zin = nc.dram_tensor("zin", [rows, DIM], BF16, kind="ExternalInput")
    mkv = nc.dram_tensor("mkv", [rows, KP], F16, kind="ExternalInput")
    tbl = nc.dram_tensor("tbl", [K, TBL_W], BF16, kind="ExternalInput")
    eye = nc.dram_tensor("eye", [P, P], BF16, kind="ExternalInput")
    out = nc.dram_tensor("out", [rows, DIM], BF16, kind="ExternalOutput")

    with tile.TileContext(nc) as tc:
        with (
            tc.tile_pool(name="zp", bufs=3) as zp,
            tc.tile_pool(name="mkp", bufs=2) as mkp,
            tc.tile_pool(name="selp", bufs=3) as selp,
            tc.tile_pool(name="outp", bufs=2) as outp,
            tc.tile_pool(name="outp", bufs=2) as outp,
            tc.tile_pool(name="scrap", bufs=4) as scrp,
            tc.tile_pool(name="ascrap", bufs=2) as ascrp,
            tc.tile_pool(name="tiny", bufs=80) as tinyp,
            tc.tile_pool(name="gacc", bufs=12) as gaccp,
        ):


            def phase1(g):
                r0, r1 = g * GRP * P, (g + 1) * GRP * P
                c0 = g * GRP
                mk_g = mkp.tile([P, GRP, KP], F16, name="mk_g", tag="mk")
                nc.sync.dma_start(
                    out=mk_g[:], in_=mkv[r0:r1].rearrange("(n p) c -> p n c", p=P)
                )
                z_g = zp.tile([P, GRP, DIM], BF16, name="z_g", tag="z")
                for n in range(GRP):
                    zr = r0 + n * P
                    nc.gpsimd.dma_start(out=z_g[:, n, :], in_=zin[zr : zr + P])
                # idx for the whole group in one cheap reduce (exact)
                nc.vector.tensor_reduce(
                    out=idxf[:, c0 : c0 + GRP],
                    in_=mk_g[:],
                    axis=mybir.AxisListType.X,
                    op=OP.add,
                )
                # convert group idx to u32 gather offsets
                nc.vector.tensor_copy(
                    out=idxu[:, c0 : c0 + GRP], in_=idxf[:, c0 : c0 + GRP]
                )
                # gather table rows (one indirect DMA per tile)
                sel = selp.tile([P, GRP, TBL_W], BF16, name="sel", tag="sel")
                for n in range(GRP):
                    nc.gpsimd.indirect_dma_start(
                        out=sel[:, n, :],
                        out_offset=None,
                        in_=tbl[:],
                        in_offset=bass.IndirectOffsetOnAxis(
                            ap=idxu[:, c0 + n : c0 + n + 1], axis=0
                        ),
                    )
                return dict(g=g, z_g=z_g, sel=sel)

            def phaseB(st):
                # t_j = z . s_j: one DVE op (elementwise mult + free-axis accum)
                g, z_g, sel = st["g"], st["z_g"], st["sel"]
                c0 = g * GRP
                for n in range(GRP):
                    j = c0 + n
                    for pole in range(2):
                        pscr = scrp.tile([P, DIM], BF16, name="pscr", tag="scr")
                        nc.vector.scalar_tensor_tensor(
                            out=pscr[:],
                            in0=z_g[:, n, :],
                            scalar=1.0,
                            in1=sel[:, n, pole * DIM : (pole + 1) * DIM],
                            op0=OP.mult,
                            op1=OP.mult,
                            accum_out=t_a[:, 2 * j + pole : 2 * j + pole + 1],
                        )

            def phase2(st, last=False):
                g = st["g"]
                r0, r1 = g * GRP * P, (g + 1) * GRP * P
                c0 = g * GRP
                z_g, sel = st["z_g"], st["sel"]
                tg = t_a[:, 8 * g : 8 * (g + 1)].rearrange(
                    "p (c t) -> p c t", t=2
                )  # [P, GRP, 2] f32
                # small per-sample math on [P, GRP(,2)] tiles
                selC = sel[:, :, PC : PC + 2]  # bf16 [P,GRP,2]
                selG2 = sel[:, :, PC + 2 : PC + 4]
                selc01 = sel[:, :, PC + 4 : PC + 5].rearrange("p c o -> p (c o)")
                u = tinyp.tile([P, GRP, 2], F32, name="u", tag="tiny")
                nc.vector.tensor_tensor(out=u[:], in0=selG2, in1=tg, op=OP.mult)
                e = tinyp.tile([P, GRP, 2], F32, name="e", tag="tiny")
                nc.scalar.activation(out=e[:], in_=u[:], func=AT.Exp)
                m = tinyp.tile([P, GRP, 2], F32, name="m", tag="tiny")
                nc.vector.tensor_tensor(out=m[:], in0=selC, in1=e[:], op=OP.mult)
                h = tinyp.tile([P, GRP, 2], F32, name="h", tag="tiny")
                nc.vector.tensor_tensor(out=h[:], in0=m[:], in1=tg, op=OP.mult)
                beta = tinyp.tile([P, GRP], F32, name="beta", tag="tiny")
                nc.vector.scalar_tensor_tensor(
                    out=beta[:], in0=h[:, :, 0], scalar=-1.0, in1=h[:, :, 1],
                    op0=OP.mult, op1=OP.subtract,
                )
                m2 = tinyp.tile([P, GRP, 2], F32, name="m2", tag="tiny")
                nc.vector.tensor_tensor(out=m2[:], in0=m[:], in1=m[:], op=OP.mult)
                s2 = tinyp.tile([P, GRP], F32, name="s2", tag="tiny")
                nc.vector.tensor_tensor(
                    out=s2[:], in0=m2[:, :, 0], in1=m2[:, :, 1], op=OP.add
                )
                mm = tinyp.tile([P, GRP], F32, name="mm", tag="tiny")
                nc.vector.tensor_tensor(
                    out=mm[:], in0=m[:, :, 0], in1=m[:, :, 1], op=OP.mult
                )
                v = tinyp.tile([P, GRP], F32, name="v", tag="tiny")
                nc.vector.tensor_tensor(out=v[:], in0=mm[:], in1=selc01, op=OP.mult)
                b2 = tinyp.tile([P, GRP], F32, name="b2", tag="tiny")
                nc.vector.tensor_tensor(out=b2[:], in0=beta[:], in1=beta[:], op=OP.mult)
                w = tinyp.tile([P, GRP], F32, name="w", tag="tiny")
                nc.vector.scalar_tensor_tensor(
                    out=w[:], in0=v[:], scalar=2.0, in1=b2[:],
                    op0=OP.mult, op1=OP.subtract,
                )
                pn = tinyp.tile([P, GRP], F32, name="pn", tag="tiny")
                nc.vector.tensor_tensor(out=pn[:], in0=s2[:], in1=w[:], op=OP.add)
                # r = 1/|p| = exp(-0.5 ln pn); ln+exp share one ACT table
                ll = tinyp.tile([P, GRP], F32, name="ll", tag="tiny")
                nc.scalar.activation(out=ll[:], in_=pn[:], func=AT.Ln)
                rr = tinyp.tile([P, GRP], F32, name="rr", tag="tiny")
                nc.scalar.activation(out=rr[:], in_=ll[:], func=AT.Exp, scale=-0.5)
                bp = tinyp.tile([P, GRP], F32, name="bp", tag="tiny")
                nc.vector.tensor_tensor(out=bp[:], in0=beta[:], in1=rr[:], op=OP.mult)
                m0p = tinyp.tile([P, GRP], F32, name="m0p", tag="tiny")
                nc.vector.tensor_tensor(
                    out=m0p[:], in0=m[:, :, 0], in1=rr[:], op=OP.mult
                )
                m1p = tinyp.tile([P, GRP], F32, name="m1p", tag="tiny")
                nc.vector.tensor_tensor(
                    out=m1p[:], in0=m[:, :, 1], in1=rr[:], op=OP.mult
                )
                # p = diag(bp) z + diag(m0p) s0 + diag(m1p) s1 on the PE,
                # accumulated in PSUM f32, copied to SBUF bf16, DMA'd out
                og = outp.tile([P, GRP, DIM], BF16, name="og", tag="og")
                for n in range(GRP):
                    dB = diagp.tile([P, P], BF16, name="dB", tag="diag")
                    nc.vector.tensor_scalar(
                        out=dB[:], in0=eyeb[:], scalar1=bp[:, n : n + 1],
                        scalar2=None, op0=OP.mult,
                    )
                    d0 = diagp.tile([P, P], BF16, name="d0", tag="diag")
                    nc.scalar.activation(
                        out=d0[:], in_=eyeb[:], func=AT.Copy,
                        scale=m0p[:, n : n + 1],
                    )
                    d1 = diagp.tile([P, P], BF16, name="d1", tag="diag")
                    nc.scalar.activation(
                        out=d1[:], in_=eyeb[:], func=AT.Copy,
                        scale=m1p[:, n : n + 1],
                    )
                    pp = psump.tile([P, 2, 512], F32, name="pp", tag="psum")
                    for h2 in range(2):
                        lo = h2 * 384
                        nc.tensor.matmul(
                            pp[:, h2, 0:384], dB[:], z_g[:, n, lo : lo + 384],
                            start=True, stop=False,
                        )
                        nc.tensor.matmul(
                            pp[:, h2, 0:384], d0[:], sel[:, n, lo : lo + 384],
                            start=False, stop=False,
                        )
                        nc.tensor.matmul(
                            pp[:, h2, 0:384], d1[:],
                            sel[:, n, DIM + lo : DIM + lo + 384],
                            start=False, stop=True,
                        )
                    nc.scalar.copy(out=og[:, n, :], in_=pp[:, :, 0:384])
                nc.sync.dma_start(
                    out=out[r0 : r0 + GRP * P].rearrange("(n p) c -> p n c", p=P),
                    in_=og[:],
                )

            sts = {}
            sts[0] = phase1(0)
            sts[1] = phase1(1)
            nc.sync.dma_start(out=eyeb[:], in_=eye[:])
            sts[2] = phase1(2)
            phaseB(sts[0])
            phase2(sts[0])
            sts[3] = phase1(3)
            phaseB(sts[1])
            phase2(sts[1])
            phaseB(sts[2])
            phase2(sts[2])
            phaseB(sts[3])
            phase2(sts[3], last=True)
    return nc


_NC_CACHE = None


def _get_nc():
    global _NC_CACHE
    if _NC_CACHE is None:
        _NC_CACHE = build_nc()
    return _NC_CACHE


def build_in_maps(inputs):
    import ml_dtypes

    z = np.asarray(inputs["z"], dtype=np.float32)
    mask = np.asarray(inputs["support_sets_mask"], dtype=np.float32)
    S = np.asarray(inputs["SUPPORT_SETS"], dtype=np.float32)
    A = np.asarray(inputs["ALPHAS"], dtype=np.float32)
    LG = np.asarray(inputs["LOGGAMMA"], dtype=np.float32)

    zb = z.astype(ml_dtypes.bfloat16)
    # scaled one-hot (value = column index), block-summed to 125 columns;
    # exact because each row has a single nonzero
    mkv = (
        (mask * np.arange(K, dtype=np.float32)[None, :])
        .reshape(BS, 8, KP)
        .sum(axis=1)
        .astype(np.float16)
    )
    # table rows: [s0 | s1 | C0 C1 2g0 2g1 c01 | pad]
    g = np.exp(LG)  # [K,2]
    C = A * g * np.exp(-2.0 * g)
    c01 = np.sum(S[:, :DIM] * S[:, DIM:], axis=1, keepdims=True)
    tblf = np.zeros((K, TBL_W), dtype=np.float32)
    tblf[:, : 2 * DIM] = S
    tblf[:, PC : PC + 2] = C
    tblf[:, PC + 2 : PC + 4] = 2.0 * g
    tblf[:, PC + 4 : PC + 5] = c01
    tbl = tblf.astype(ml_dtypes.bfloat16)
    eye = np.eye(P, dtype=np.float32).astype(ml_dtypes.bfloat16)

    return [
        {
            "zin": np.ascontiguousarray(zb[c * ROWS : (c + 1) * ROWS]),
            "mkv": np.ascontiguousarray(mkv[c * ROWS : (c + 1) * ROWS]),
            "tbl": tbl,
            "eye": eye,
        }
        for c in range(NCORES)
    ]


def kernel(support_sets_mask, z, SUPPORT_SETS, ALPHAS, LOGGAMMA):
    in_maps = build_in_maps(
        dict(
            support_sets_mask=support_sets_mask, z=z,
            SUPPORT_SETS=SUPPORT_SETS, ALPHAS=ALPHAS, LOGGAMMA=LOGGAMMA,
        )
    )
    nc = _get_nc()
    res = run_bass_kernel_spmd(nc, in_maps, list(range(NCORES)))
    return np.concatenate(
        [res.results[c]["out"] for c in range(NCORES)], axis=0
    ).astype(np.float32)


# revision 34
# speedup vs baseline: 1.2267x; 1.0347x over previous
"""Trainium2 Bass kernel for CorpusSupportSets RBF tangent-field.

Math per sample row i (dim 768), one-hot mask selects dipole k:
    t_j  = z . s_j                      (unit z, unit s_j)
    m_j  = a_j g_j e^{-g_j(2-2t_j)} = C_j exp(2 g_j t_j),  C_j = a_j g_j e^{-2 g_j}
    beta = -(m0 t0 + m1 t1)
    p    = beta z + m0 s0 + m1 s1
    |p|^2 = m0^2 + m1^2 - beta^2 + 2 m0 m1 (s0.s1)
    out  = p / |p|

Sharding: data-parallel over batch across 8 cores (2048 rows each).
Host prep (dtype/layout only + per-table-row constants): z -> bf16; the
one-hot mask scaled by column index is block-summed to 125 f16 columns
(exact: one nonzero per row), so idx is a cheap on-device reduce; table
rows [s0|s1|C0 C1 2g0 2g1 c01|pad] in bf16; output computed in bf16 and
upcast to f32 on host.

Per-sample row selection uses indirect DMA row gathers with u32
per-partition offsets. The dots t_j are single DVE ops (stt with
accum_out). Phase 2 (p = bp z + m0p s0 + m1p s1, coefficients
pre-scaled by 1/|p| = exp(-0.5 ln pn), keeping ACT on one table) runs
on the otherwise-idle Tensor engine as accumulating matmuls with
diagonal stationary matrices diag(coef) = eye * coef.
"""
import sys

for _p in ("/opt/trn_rl_repo",):
    if _p not in sys.path:
        sys.path.insert(0, _p)

import numpy as np

import concourse.bass as bass
import concourse.tile as tile
from concourse import mybir
from concourse.bass_utils import run_bass_kernel_spmd
from concourse.vector_clock import ScopedClock

# ---------------------------------------------------------------------------
# Workaround: this walrus build only accepts ONE semaphore wait per
# instruction; the TileContext exit drain accumulates one wait per live
# semaphore lane.  Split overflow waits onto trailing sync-engine NOPs.
_MAX_WAITS = 1


def _split_waits(nc, inst):
    si = inst.sync_info
    if si is None:
        return
    waits = list(si.on_wait)
    if len(waits) <= _MAX_WAITS:
        return
    inst.sync_info = mybir.SyncInfo(
        on_wait=waits[:_MAX_WAITS], on_update=list(si.on_update)
    )
    for i in range(_MAX_WAITS, len(waits), _MAX_WAITS):
        nop = nc.sync.nop(nofuse=True, hint="drain_wait_overflow")
        nop.ins.sync_info = mybir.SyncInfo(
            on_wait=waits[i : i + _MAX_WAITS], on_update=[]
        )


def _patched_drain_and_barrier(self, tick_clock, wait_clock):
    drain_inst = self.nc.sync.drain()
    wait_clock.add_sem_waits(
        drain_inst.ins, ScopedClock({None: tick_clock.global_clock})
    )
    _split_waits(self.nc, drain_inst.ins)
    self.nc.all_engine_barrier()
    assert self.sems is not None
    popped = self.nc._tile_sem_poison_stack.pop()
    assert popped is self._sem_poison
    self.nc.clear_and_free_semaphores(list(self.sems.allocated().values()))
    self.nc.all_engine_barrier()


_orig_commit = tile.TileContext._commit_instruction


def _patched_commit(self, inst, lazy_reg_writes=True):
    si = getattr(inst, "sync_info", None)
    if (
        si is not None
        and si.on_wait
        and len(si.on_wait) > _MAX_WAITS
        and inst.engine != mybir.EngineType.Unassigned
    ):
        waits = list(si.on_wait)
        inst.sync_info = mybir.SyncInfo(
            on_wait=waits[:_MAX_WAITS], on_update=list(si.on_update)
        )
        for _i, _w in enumerate(waits[_MAX_WAITS:]):
            nop = mybir.InstNoOp(
                name=f"{inst.name}_w{_i}",
                engine=inst.engine,
                sync_info=mybir.SyncInfo(on_wait=[_w], on_update=[]),
                bass_nofuse=True,
            )
            self._add_instruction(nop)
    return _orig_commit(self, inst, lazy_reg_writes)


tile.TileContext._drain_and_barrier = _patched_drain_and_barrier
tile.TileContext._commit_instruction = _patched_commit

# ---------------------------------------------------------------------------
BS, K, DIM = 16384, 1000, 768
NCORES = 8
ROWS = BS // NCORES  # 2048 rows per core
P = 128
NT = ROWS // P  # 16 tiles of 128 rows
GRP = 4  # tiles per group
NG = NT // GRP  # 4 groups
KP = 125  # packed (block-summed) scaled-mask width: 1000 = 8 * 125
TBL_W = 2 * DIM + 8  # 1544 bf16 cols = 3088B per row
# param columns inside a table row
PC = 2 * DIM  # C0, C1, 2g0, 2g1, c01 start here
F32 = mybir.dt.float32
BF16 = mybir.dt.bfloat16
F16 = mybir.dt.float16
U32 = mybir.dt.uint32


def build_nc(rows=ROWS):
    NT = rows // P
    NG = NT // GRP
    OP = mybir.AluOpType
    AT = mybir.ActivationFunctionType
    nc = bass.Bass()
    zin = nc.dram_tensor("zin", [rows, DIM], BF16, kind="ExternalInput")
    mkv = nc.dram_tensor("mkv", [rows, KP], F16, kind="ExternalInput")
    tbl = nc.dram_tensor("tbl", [K, TBL_W], BF16, kind="ExternalInput")
    eye = nc.dram_tensor("eye", [P, P], BF16, kind="ExternalInput")
    out = nc.dram_tensor("out", [rows, DIM], BF16, kind="ExternalOutput")

    with tile.TileContext(nc) as tc:
        with (
            tc.tile_pool(name="zp", bufs=3) as zp,
            tc.tile_pool(name="mkp", bufs=2) as mkp,
            tc.tile_pool(name="selp", bufs=3) as selp,
            tc.tile_pool(name="outp", bufs=2) as outp,
            tc.tile_pool(name="outp", bufs=2) as outp,
            tc.tile_pool(name="scrap", bufs=4) as scrp,
            tc.tile_pool(name="ascrap", bufs=2) as ascrp,
            tc.tile_pool(name="tiny", bufs=80) as tinyp,
            tc.tile_pool(name="gacc", bufs=12) as gaccp,
        ):


            def phase1(g):
                r0, r1 = g * GRP * P, (g + 1) * GRP * P
                c0 = g * GRP
                mk_g = mkp.tile([P, GRP, KP], F16, name="mk_g", tag="mk")
                nc.sync.dma_start(
                    out=mk_g[:], in_=mkv[r0:r1].rearrange("(n p) c -> p n c", p=P)
                )
                z_g = zp.tile([P, GRP, DIM], BF16, name="z_g", tag="z")
                for n in range(GRP):
                    zr = r0 + n * P
                    nc.gpsimd.dma_start(out=z_g[:, n, :], in_=zin[zr : zr + P])
                # idx for the whole group in one cheap reduce (exact)
                nc.vector.tensor_reduce(
                    out=idxf[:, c0 : c0 + GRP],
                    in_=mk_g[:],
                    axis=mybir.AxisListType.X,
                    op=OP.add,
                )
                # convert group idx to u32 gather offsets
                nc.vector.tensor_copy(
                    out=idxu[:, c0 : c0 + GRP], in_=idxf[:, c0 : c0 + GRP]
                )
                # gather table rows (one indirect DMA per tile)
                sel = selp.tile([P, GRP, TBL_W], BF16, name="sel", tag="sel")
                for n in range(GRP):
                    nc.gpsimd.indirect_dma_start(
                        out=sel[:, n, :],
                        out_offset=None,
                        in_=tbl[:],
                        in_offset=bass.IndirectOffsetOnAxis(
                            ap=idxu[:, c0 + n : c0 + n + 1], axis=0
                        ),
                    )
                return dict(g=g, z_g=z_g, sel=sel)

            def phaseB(st):
                # t_j = z . s_j: one DVE op (elementwise mult + free-axis accum)
                g, z_g, sel = st["g"], st["z_g"], st["sel"]
                c0 = g * GRP
                for n in range(GRP):
                    j = c0 + n
                    for pole in range(2):
                        pscr = scrp.tile([P, DIM], BF16, name="pscr", tag="scr")
                        nc.vector.scalar_tensor_tensor(
                            out=pscr[:],
                            in0=z_g[:, n, :],
                            scalar=1.0,
                            in1=sel[:, n, pole * DIM : (pole + 1) * DIM],
                            op0=OP.mult,
                            op1=OP.mult,
                            accum_out=t_a[:, 2 * j + pole : 2 * j + pole + 1],
                        )

            def phase2(st, last=False):
                g = st["g"]
                r0, r1 = g * GRP * P, (g + 1) * GRP * P
                c0 = g * GRP
                z_g, sel = st["z_g"], st["sel"]
                tg = t_a[:, 8 * g : 8 * (g + 1)].rearrange(
                    "p (c t) -> p c t", t=2
                )  # [P, GRP, 2] f32
                # small per-sample math on [P, GRP(,2)] tiles
                selC = sel[:, :, PC : PC + 2]  # bf16 [P,GRP,2]
                selG2 = sel[:, :, PC + 2 : PC + 4]
                selc01 = sel[:, :, PC + 4 : PC + 5].rearrange("p c o -> p (c o)")
                u = tinyp.tile([P, GRP, 2], F32, name="u", tag="tiny")
                nc.vector.tensor_tensor(out=u[:], in0=selG2, in1=tg, op=OP.mult)
                e = tinyp.tile([P, GRP, 2], F32, name="e", tag="tiny")
                nc.scalar.activation(out=e[:], in_=u[:], func=AT.Exp)
                m = tinyp.tile([P, GRP, 2], F32, name="m", tag="tiny")
                nc.vector.tensor_tensor(out=m[:], in0=selC, in1=e[:], op=OP.mult)
                h = tinyp.tile([P, GRP, 2], F32, name="h", tag="tiny")
                nc.vector.tensor_tensor(out=h[:], in0=m[:], in1=tg, op=OP.mult)
                beta = tinyp.tile([P, GRP], F32, name="beta", tag="tiny")
                nc.vector.scalar_tensor_tensor(
                    out=beta[:], in0=h[:, :, 0], scalar=-1.0, in1=h[:, :, 1],
                    op0=OP.mult, op1=OP.subtract,
                )
                m2 = tinyp.tile([P, GRP, 2], F32, name="m2", tag="tiny")
                nc.vector.tensor_tensor(out=m2[:], in0=m[:], in1=m[:], op=OP.mult)
                s2 = tinyp.tile([P, GRP], F32, name="s2", tag="tiny")
                nc.vector.tensor_tensor(
                    out=s2[:], in0=m2[:, :, 0], in1=m2[:, :, 1], op=OP.add
                )
                mm = tinyp.tile([P, GRP], F32, name="mm", tag="tiny")
                nc.vector.tensor_tensor(
                    out=mm[:], in0=m[:, :, 0], in1=m[:, :, 1], op=OP.mult
                )
                v = tinyp.tile([P, GRP], F32, name="v", tag="tiny")
                nc.vector.tensor_tensor(out=v[:], in0=mm[:], in1=selc01, op=OP.mult)
                b2 = tinyp.tile([P, GRP], F32, name="b2", tag="tiny")
                nc.vector.tensor_tensor(out=b2[:], in0=beta[:], in1=beta[:], op=OP.mult)
                w = tinyp.tile([P, GRP], F32, name="w", tag="tiny")
                nc.vector.scalar_tensor_tensor(
                    out=w[:], in0=v[:], scalar=2.0, in1=b2[:],
                    op0=OP.mult, op1=OP.subtract,
                )
                pn = tinyp.tile([P, GRP], F32, name="pn", tag="tiny")
                nc.vector.tensor_tensor(out=pn[:], in0=s2[:], in1=w[:], op=OP.add)
                # r = 1/|p| = exp(-0.5 ln pn); ln+exp share one ACT table
                ll = tinyp.tile([P, GRP], F32, name="ll", tag="tiny")
                nc.scalar.activation(out=ll[:], in_=pn[:], func=AT.Ln)
                rr = tinyp.tile([P, GRP], F32, name="rr", tag="tiny")
                nc.scalar.activation(out=rr[:], in_=ll[:], func=AT.Exp, scale=-0.5)
                bp = tinyp.tile([P, GRP], F32, name="bp", tag="tiny")
                nc.vector.tensor_tensor(out=bp[:], in0=beta[:], in1=rr[:], op=OP.mult)
                m0p = tinyp.tile([P, GRP], F32, name="m0p", tag="tiny")
                nc.vector.tensor_tensor(
                    out=m0p[:], in0=m[:, :, 0], in1=rr[:], op=OP.mult
                )
                m1p = tinyp.tile([P, GRP], F32, name="m1p", tag="tiny")
                nc.vector.tensor_tensor(
                    out=m1p[:], in0=m[:, :, 1], in1=rr[:], op=OP.mult
                )
                # p = diag(bp) z + diag(m0p) s0 + diag(m1p) s1 on the PE,
                # accumulated in PSUM f32, copied to SBUF bf16, DMA'd out
                og = outp.tile([P, GRP, DIM], BF16, name="og", tag="og")
                for n in range(GRP):
                    dB = diagp.tile([P, P], BF16, name="dB", tag="diag")
                    nc.vector.tensor_scalar(
                        out=dB[:], in0=eyeb[:], scalar1=bp[:, n : n + 1],
                        scalar2=None, op0=OP.mult,
                    )
                    d0 = diagp.tile([P, P], BF16, name="d0", tag="diag")
                    nc.scalar.activation(
                        out=d0[:], in_=eyeb[:], func=AT.Copy,
                        scale=m0p[:, n : n + 1],
                    )
                    d1 = diagp.tile([P, P], BF16, name="d1", tag="diag")
                    nc.scalar.activation(
                        out=d1[:], in_=eyeb[:], func=AT.Copy,
                        scale=m1p[:, n : n + 1],
                    )
                    pp = psump.tile([P, 2, 512], F32, name="pp", tag="psum")
                    for h2 in range(2):
                        lo = h2 * 384
                        nc.tensor.matmul(
                            pp[:, h2, 0:384], dB[:], z_g[:, n, lo : lo + 384],
                            start=True, stop=False,
                        )
                        nc.tensor.matmul(
                            pp[:, h2, 0:384], d0[:], sel[:, n, lo : lo + 384],
                            start=False, stop=False,
                        )
                        nc.tensor.matmul(
                            pp[:, h2, 0:384], d1[:],
                            sel[:, n, DIM + lo : DIM + lo + 384],
                            start=False, stop=True,
                        )
                    nc.scalar.copy(out=og[:, n, :], in_=pp[:, :, 0:384])
                nc.sync.dma_start(
                    out=out[r0 : r0 + GRP * P].rearrange("(n p) c -> p n c", p=P),
                    in_=og[:],
                )

            sts = {}
            sts[0] = phase1(0)
            sts[1] = phase1(1)
            nc.sync.dma_start(out=eyeb[:], in_=eye[:])
            sts[2] = phase1(2)
            phaseB(sts[0])
            phase2(sts[0])
            sts[3] = phase1(3)
            phaseB(sts[1])
            phase2(sts[1])
            phaseB(sts[2])
            phase2(sts[2])
            phaseB(sts[3])
            phase2(sts[3], last=True)
    return nc


_NC_CACHE = None


def _get_nc():
    global _NC_CACHE
    if _NC_CACHE is None:
        _NC_CACHE = build_nc()
    return _NC_CACHE


def build_in_maps(inputs):
    import ml_dtypes

    z = np.asarray(inputs["z"], dtype=np.float32)
    mask = np.asarray(inputs["support_sets_mask"], dtype=np.float32)
    S = np.asarray(inputs["SUPPORT_SETS"], dtype=np.float32)
    A = np.asarray(inputs["ALPHAS"], dtype=np.float32)
    LG = np.asarray(inputs["LOGGAMMA"], dtype=np.float32)

    zb = z.astype(ml_dtypes.bfloat16)
    # scaled one-hot (value = column index), block-summed to 125 columns;
    # exact because each row has a single nonzero
    mkv = (
        (mask * np.arange(K, dtype=np.float32)[None, :])
        .reshape(BS, 8, KP)
        .sum(axis=1)
        .astype(np.float16)
    )
    # table rows: [s0 | s1 | C0 C1 2g0 2g1 c01 | pad]
    g = np.exp(LG)  # [K,2]
    C = A * g * np.exp(-2.0 * g)
    c01 = np.sum(S[:, :DIM] * S[:, DIM:], axis=1, keepdims=True)
    tblf = np.zeros((K, TBL_W), dtype=np.float32)
    tblf[:, : 2 * DIM] = S
    tblf[:, PC : PC + 2] = C
    tblf[:, PC + 2 : PC + 4] = 2.0 * g
    tblf[:, PC + 4 : PC + 5] = c01
    tbl = tblf.astype(ml_dtypes.bfloat16)
    eye = np.eye(P, dtype=np.float32).astype(ml_dtypes.bfloat16)

    return [
        {
            "zin": np.ascontiguousarray(zb[c * ROWS : (c + 1) * ROWS]),
            "mkv": np.ascontiguousarray(mkv[c * ROWS : (c + 1) * ROWS]),
            "tbl": tbl,
            "eye": eye,
        }
        for c in range(NCORES)
    ]


def kernel(support_sets_mask, z, SUPPORT_SETS, ALPHAS, LOGGAMMA):
    in_maps = build_in_maps(
        dict(
            support_sets_mask=support_sets_mask, z=z,
            SUPPORT_SETS=SUPPORT_SETS, ALPHAS=ALPHAS, LOGGAMMA=LOGGAMMA,
        )
    )
    nc = _get_nc()
    res = run_bass_kernel_spmd(nc, in_maps, list(range(NCORES)))
    return np.concatenate(
        [res.results[c]["out"] for c in range(NCORES)], axis=0
    ).astype(np.float32)


# revision 35
# speedup vs baseline: 1.2599x; 1.0271x over previous
"""Trainium2 Bass kernel for CorpusSupportSets RBF tangent-field.

Math per sample row i (dim 768), one-hot mask selects dipole k:
    t_j  = z . s_j                      (unit z, unit s_j)
    m_j  = a_j g_j e^{-g_j(2-2t_j)} = C_j exp(2 g_j t_j),  C_j = a_j g_j e^{-2 g_j}
    beta = -(m0 t0 + m1 t1)
    p    = beta z + m0 s0 + m1 s1
    |p|^2 = m0^2 + m1^2 - beta^2 + 2 m0 m1 (s0.s1)
    out  = p / |p|

Sharding: data-parallel over batch across 8 cores (2048 rows each).
Host prep (dtype/layout only + per-table-row constants): z -> bf16; the
one-hot mask scaled by column index is block-summed to 125 f16 columns
(exact: one nonzero per row), so idx is a cheap on-device reduce; table
rows [s0|s1|C0 C1 2g0 2g1 c01|pad] in bf16; output computed in bf16 and
upcast to f32 on host.

Per-sample row selection uses indirect DMA row gathers with u32
per-partition offsets. The dots t_j are single DVE ops (stt with
accum_out). Phase 2 (p = bp z + m0p s0 + m1p s1, coefficients
pre-scaled by 1/|p| = exp(-0.5 ln pn), keeping ACT on one table) runs
on the otherwise-idle Tensor engine as accumulating matmuls with
diagonal stationary matrices diag(coef) = eye * coef.
"""
import sys

for _p in ("/opt/trn_rl_repo",):
    if _p not in sys.path:
        sys.path.insert(0, _p)

import numpy as np

import concourse.bass as bass
import concourse.tile as tile
from concourse import mybir
from concourse.bass_utils import run_bass_kernel_spmd
from concourse.vector_clock import ScopedClock

# ---------------------------------------------------------------------------
# Workaround: this walrus build only accepts ONE semaphore wait per
# instruction; the TileContext exit drain accumulates one wait per live
# semaphore lane.  Split overflow waits onto trailing sync-engine NOPs.
_MAX_WAITS = 1


def _split_waits(nc, inst):
    si = inst.sync_info
    if si is None:
        return
    waits = list(si.on_wait)
    if len(waits) <= _MAX_WAITS:
        return
    inst.sync_info = mybir.SyncInfo(
        on_wait=waits[:_MAX_WAITS], on_update=list(si.on_update)
    )
    for i in range(_MAX_WAITS, len(waits), _MAX_WAITS):
        nop = nc.sync.nop(nofuse=True, hint="drain_wait_overflow")
        nop.ins.sync_info = mybir.SyncInfo(
            on_wait=waits[i : i + _MAX_WAITS], on_update=[]
        )


def _patched_drain_and_barrier(self, tick_clock, wait_clock):
    drain_inst = self.nc.sync.drain()
    wait_clock.add_sem_waits(
        drain_inst.ins, ScopedClock({None: tick_clock.global_clock})
    )
    _split_waits(self.nc, drain_inst.ins)
    self.nc.all_engine_barrier()
    assert self.sems is not None
    popped = self.nc._tile_sem_poison_stack.pop()
    assert popped is self._sem_poison
    self.nc.clear_and_free_semaphores(list(self.sems.allocated().values()))
    self.nc.all_engine_barrier()


_orig_commit = tile.TileContext._commit_instruction


def _patched_commit(self, inst, lazy_reg_writes=True):
    si = getattr(inst, "sync_info", None)
    if (
        si is not None
        and si.on_wait
        and len(si.on_wait) > _MAX_WAITS
        and inst.engine != mybir.EngineType.Unassigned
    ):
        waits = list(si.on_wait)
        inst.sync_info = mybir.SyncInfo(
            on_wait=waits[:_MAX_WAITS], on_update=list(si.on_update)
        )
        for _i, _w in enumerate(waits[_MAX_WAITS:]):
            nop = mybir.InstNoOp(
                name=f"{inst.name}_w{_i}",
                engine=inst.engine,
                sync_info=mybir.SyncInfo(on_wait=[_w], on_update=[]),
                bass_nofuse=True,
            )
            self._add_instruction(nop)
    return _orig_commit(self, inst, lazy_reg_writes)


tile.TileContext._drain_and_barrier = _patched_drain_and_barrier
tile.TileContext._commit_instruction = _patched_commit

# ---------------------------------------------------------------------------
BS, K, DIM = 16384, 1000, 768
NCORES = 8
ROWS = BS // NCORES  # 2048 rows per core
P = 128
NT = ROWS // P  # 16 tiles of 128 rows
GRP = 4  # tiles per group
NG = NT // GRP  # 4 groups
KP = 125  # packed (block-summed) scaled-mask width: 1000 = 8 * 125
TBL_W = 2 * DIM + 8  # 1544 bf16 cols = 3088B per row
# param columns inside a table row
PC = 2 * DIM  # C0, C1, 2g0, 2g1, c01 start here
F32 = mybir.dt.float32
BF16 = mybir.dt.bfloat16
F16 = mybir.dt.float16
U32 = mybir.dt.uint32


def build_nc(rows=ROWS):
    NT = rows // P
    NG = NT // GRP
    OP = mybir.AluOpType
    AT = mybir.ActivationFunctionType
    nc = bass.Bass()
    zin = nc.dram_tensor("zin", [rows, DIM], BF16, kind="ExternalInput")
    mkv = nc.dram_tensor("mkv", [rows, KP], F16, kind="ExternalInput")
    tbl = nc.dram_tensor("tbl", [K, TBL_W], BF16, kind="ExternalInput")
    eye = nc.dram_tensor("eye", [P, P], BF16, kind="ExternalInput")
    out = nc.dram_tensor("out", [rows, DIM], BF16, kind="ExternalOutput")

    with tile.TileContext(nc) as tc:
        with (
            tc.tile_pool(name="zp", bufs=3) as zp,
            tc.tile_pool(name="mkp", bufs=2) as mkp,
            tc.tile_pool(name="selp", bufs=4) as selp,
            tc.tile_pool(name="outp", bufs=2) as outp,
            tc.tile_pool(name="outp", bufs=2) as outp,
            tc.tile_pool(name="scrap", bufs=4) as scrp,
            tc.tile_pool(name="ascrap", bufs=2) as ascrp,
            tc.tile_pool(name="tiny", bufs=80) as tinyp,
            tc.tile_pool(name="gacc", bufs=12) as gaccp,
        ):


            def phase1(g):
                r0, r1 = g * GRP * P, (g + 1) * GRP * P
                c0 = g * GRP
                mk_g = mkp.tile([P, GRP, KP], F16, name="mk_g", tag="mk")
                nc.sync.dma_start(
                    out=mk_g[:], in_=mkv[r0:r1].rearrange("(n p) c -> p n c", p=P)
                )
                z_g = zp.tile([P, GRP, DIM], BF16, name="z_g", tag="z")
                for n in range(GRP):
                    zr = r0 + n * P
                    nc.gpsimd.dma_start(out=z_g[:, n, :], in_=zin[zr : zr + P])
                # idx for the whole group in one cheap reduce (exact)
                nc.vector.tensor_reduce(
                    out=idxf[:, c0 : c0 + GRP],
                    in_=mk_g[:],
                    axis=mybir.AxisListType.X,
                    op=OP.add,
                )
                # convert group idx to u32 gather offsets
                nc.vector.tensor_copy(
                    out=idxu[:, c0 : c0 + GRP], in_=idxf[:, c0 : c0 + GRP]
                )
                # gather table rows (one indirect DMA per tile)
                sel = selp.tile([P, GRP, TBL_W], BF16, name="sel", tag="sel")
                for n in range(GRP):
                    nc.gpsimd.indirect_dma_start(
                        out=sel[:, n, :],
                        out_offset=None,
                        in_=tbl[:],
                        in_offset=bass.IndirectOffsetOnAxis(
                            ap=idxu[:, c0 + n : c0 + n + 1], axis=0
                        ),
                    )
                return dict(g=g, z_g=z_g, sel=sel)

            def phaseB(st):
                # t_j = z . s_j: one DVE op (elementwise mult + free-axis accum)
                g, z_g, sel = st["g"], st["z_g"], st["sel"]
                c0 = g * GRP
                for n in range(GRP):
                    j = c0 + n
                    for pole in range(2):
                        pscr = scrp.tile([P, DIM], BF16, name="pscr", tag="scr")
                        nc.vector.scalar_tensor_tensor(
                            out=pscr[:],
                            in0=z_g[:, n, :],
                            scalar=1.0,
                            in1=sel[:, n, pole * DIM : (pole + 1) * DIM],
                            op0=OP.mult,
                            op1=OP.mult,
                            accum_out=t_a[:, 2 * j + pole : 2 * j + pole + 1],
                        )

            def phase2(st, last=False):
                g = st["g"]
                r0, r1 = g * GRP * P, (g + 1) * GRP * P
                c0 = g * GRP
                z_g, sel = st["z_g"], st["sel"]
                tg = t_a[:, 8 * g : 8 * (g + 1)].rearrange(
                    "p (c t) -> p c t", t=2
                )  # [P, GRP, 2] f32
                # small per-sample math on [P, GRP(,2)] tiles
                selC = sel[:, :, PC : PC + 2]  # bf16 [P,GRP,2]
                selG2 = sel[:, :, PC + 2 : PC + 4]
                selc01 = sel[:, :, PC + 4 : PC + 5].rearrange("p c o -> p (c o)")
                u = tinyp.tile([P, GRP, 2], F32, name="u", tag="tiny")
                nc.vector.tensor_tensor(out=u[:], in0=selG2, in1=tg, op=OP.mult)
                e = tinyp.tile([P, GRP, 2], F32, name="e", tag="tiny")
                nc.scalar.activation(out=e[:], in_=u[:], func=AT.Exp)
                m = tinyp.tile([P, GRP, 2], F32, name="m", tag="tiny")
                nc.vector.tensor_tensor(out=m[:], in0=selC, in1=e[:], op=OP.mult)
                h = tinyp.tile([P, GRP, 2], F32, name="h", tag="tiny")
                nc.vector.tensor_tensor(out=h[:], in0=m[:], in1=tg, op=OP.mult)
                beta = tinyp.tile([P, GRP], F32, name="beta", tag="tiny")
                nc.vector.scalar_tensor_tensor(
                    out=beta[:], in0=h[:, :, 0], scalar=-1.0, in1=h[:, :, 1],
                    op0=OP.mult, op1=OP.subtract,
                )
                m2 = tinyp.tile([P, GRP, 2], F32, name="m2", tag="tiny")
                nc.vector.tensor_tensor(out=m2[:], in0=m[:], in1=m[:], op=OP.mult)
                s2 = tinyp.tile([P, GRP], F32, name="s2", tag="tiny")
                nc.vector.tensor_tensor(
                    out=s2[:], in0=m2[:, :, 0], in1=m2[:, :, 1], op=OP.add
                )
                mm = tinyp.tile([P, GRP], F32, name="mm", tag="tiny")
                nc.vector.tensor_tensor(
                    out=mm[:], in0=m[:, :, 0], in1=m[:, :, 1], op=OP.mult
                )
                v = tinyp.tile([P, GRP], F32, name="v", tag="tiny")
                nc.vector.tensor_tensor(out=v[:], in0=mm[:], in1=selc01, op=OP.mult)
                b2 = tinyp.tile([P, GRP], F32, name="b2", tag="tiny")
                nc.vector.tensor_tensor(out=b2[:], in0=beta[:], in1=beta[:], op=OP.mult)
                w = tinyp.tile([P, GRP], F32, name="w", tag="tiny")
                nc.vector.scalar_tensor_tensor(
                    out=w[:], in0=v[:], scalar=2.0, in1=b2[:],
                    op0=OP.mult, op1=OP.subtract,
                )
                pn = tinyp.tile([P, GRP], F32, name="pn", tag="tiny")
                nc.vector.tensor_tensor(out=pn[:], in0=s2[:], in1=w[:], op=OP.add)
                # r = 1/|p| = exp(-0.5 ln pn); ln+exp share one ACT table
                ll = tinyp.tile([P, GRP], F32, name="ll", tag="tiny")
                nc.scalar.activation(out=ll[:], in_=pn[:], func=AT.Ln)
                rr = tinyp.tile([P, GRP], F32, name="rr", tag="tiny")
                nc.scalar.activation(out=rr[:], in_=ll[:], func=AT.Exp, scale=-0.5)
                bp = tinyp.tile([P, GRP], F32, name="bp", tag="tiny")
                nc.vector.tensor_tensor(out=bp[:], in0=beta[:], in1=rr[:], op=OP.mult)
                m0p = tinyp.tile([P, GRP], F32, name="m0p", tag="tiny")
                nc.vector.tensor_tensor(
                    out=m0p[:], in0=m[:, :, 0], in1=rr[:], op=OP.mult
                )
                m1p = tinyp.tile([P, GRP], F32, name="m1p", tag="tiny")
                nc.vector.tensor_tensor(
                    out=m1p[:], in0=m[:, :, 1], in1=rr[:], op=OP.mult
                )
                # p = diag(bp) z + diag(m0p) s0 + diag(m1p) s1 on the PE,
                # accumulated in PSUM f32, copied to SBUF bf16, DMA'd out
                og = outp.tile([P, GRP, DIM], BF16, name="og", tag="og")
                for n in range(GRP):
                    dB = diagp.tile([P, P], BF16, name="dB", tag="diag")
                    nc.vector.tensor_scalar(
                        out=dB[:], in0=eyeb[:], scalar1=bp[:, n : n + 1],
                        scalar2=None, op0=OP.mult,
                    )
                    d0 = diagp.tile([P, P], BF16, name="d0", tag="diag")
                    nc.scalar.activation(
                        out=d0[:], in_=eyeb[:], func=AT.Copy,
                        scale=m0p[:, n : n + 1],
                    )
                    d1 = diagp.tile([P, P], BF16, name="d1", tag="diag")
                    nc.scalar.activation(
                        out=d1[:], in_=eyeb[:], func=AT.Copy,
                        scale=m1p[:, n : n + 1],
                    )
                    pp = psump.tile([P, 2, 512], F32, name="pp", tag="psum")
                    for h2 in range(2):
                        lo = h2 * 384
                        nc.tensor.matmul(
                            pp[:, h2, 0:384], dB[:], z_g[:, n, lo : lo + 384],
                            start=True, stop=False,
                        )
                        nc.tensor.matmul(
                            pp[:, h2, 0:384], d0[:], sel[:, n, lo : lo + 384],
                            start=False, stop=False,
                        )
                        nc.tensor.matmul(
                            pp[:, h2, 0:384], d1[:],
                            sel[:, n, DIM + lo : DIM + lo + 384],
                            start=False, stop=True,
                        )
                    nc.scalar.copy(out=og[:, n, :], in_=pp[:, :, 0:384])
                nc.sync.dma_start(
                    out=out[r0 : r0 + GRP * P].rearrange("(n p) c -> p n c", p=P),
                    in_=og[:],
                )

            sts = {}
            sts[0] = phase1(0)
            sts[1] = phase1(1)
            nc.sync.dma_start(out=eyeb[:], in_=eye[:])
            sts[2] = phase1(2)
            phaseB(sts[0])
            phase2(sts[0])
            sts[3] = phase1(3)
            phaseB(sts[1])
            phase2(sts[1])
            phaseB(sts[2])
            phase2(sts[2])
            phaseB(sts[3])
            phase2(sts[3], last=True)
    return nc


_NC_CACHE = None


def _get_nc():
    global _NC_CACHE
    if _NC_CACHE is None:
        _NC_CACHE = build_nc()
    return _NC_CACHE


def build_in_maps(inputs):
    import ml_dtypes

    z = np.asarray(inputs["z"], dtype=np.float32)
    mask = np.asarray(inputs["support_sets_mask"], dtype=np.float32)
    S = np.asarray(inputs["SUPPORT_SETS"], dtype=np.float32)
    A = np.asarray(inputs["ALPHAS"], dtype=np.float32)
    LG = np.asarray(inputs["LOGGAMMA"], dtype=np.float32)

    zb = z.astype(ml_dtypes.bfloat16)
    # scaled one-hot (value = column index), block-summed to 125 columns;
    # exact because each row has a single nonzero
    mkv = (
        (mask * np.arange(K, dtype=np.float32)[None, :])
        .reshape(BS, 8, KP)
        .sum(axis=1)
        .astype(np.float16)
    )
    # table rows: [s0 | s1 | C0 C1 2g0 2g1 c01 | pad]
    g = np.exp(LG)  # [K,2]
    C = A * g * np.exp(-2.0 * g)
    c01 = np.sum(S[:, :DIM] * S[:, DIM:], axis=1, keepdims=True)
    tblf = np.zeros((K, TBL_W), dtype=np.float32)
    tblf[:, : 2 * DIM] = S
    tblf[:, PC : PC + 2] = C
    tblf[:, PC + 2 : PC + 4] = 2.0 * g
    tblf[:, PC + 4 : PC + 5] = c01
    tbl = tblf.astype(ml_dtypes.bfloat16)
    eye = np.eye(P, dtype=np.float32).astype(ml_dtypes.bfloat16)

    return [
        {
            "zin": np.ascontiguousarray(zb[c * ROWS : (c + 1) * ROWS]),
            "mkv": np.ascontiguousarray(mkv[c * ROWS : (c + 1) * ROWS]),
            "tbl": tbl,
            "eye": eye,
        }
        for c in range(NCORES)
    ]


def kernel(support_sets_mask, z, SUPPORT_SETS, ALPHAS, LOGGAMMA):
    in_maps = build_in_maps(
        dict(
            support_sets_mask=support_sets_mask, z=z,
            SUPPORT_SETS=SUPPORT_SETS, ALPHAS=ALPHAS, LOGGAMMA=LOGGAMMA,
        )
    )
    nc = _get_nc()
    res = run_bass_kernel_spmd(nc, in_maps, list(range(NCORES)))
    return np.concatenate(
        [res.results[c]["out"] for c in range(NCORES)], axis=0
    ).astype(np.float32)
